# revision 15
# baseline (speedup 1.0000x reference)
"""CSWin self-attention Trainium2 kernel (optimized).

Sharding: data-parallel over batch B=8 across 8 cores (1 image per core).

Per-core pipeline (image = 128x128 spatial, C=256):
  A) LayerNorm (gamma/beta folded into Wqkv on host): batched x loads
     (4 token-tiles per DMA), bn_stats/aggr per tile, rstd via
     ACT ln/exp (same table set as attention's exp -> no table thrash),
     apply on DVE, PE-transpose to channel-major y^T, paired evac.
  B) Horizontal stripes overlap phase A (stripe s needs row tiles
     2s,2s+1 only). Vertical stripes after. Per stripe (seq 256,
     4 heads x head_dim 32):
       qkv matmuls into one PSUM tile (q^T,k^T ch-major; v token-major),
       S^T row-tiled 4 heads (K=32), exp on ACT (scale folded),
       attn@V col-tiled head pairs (M=64 with ones cols -> denominators
       in rows 32/96), o evac to a per-8-stripe staging block.
     Per 8-stripe block: 8 SWDGE (gpsimd) compaction DMAs with 3D APs
     (4 for O^T bands, 4 for denominator bands), reciprocal on DVE,
     normalize multiply on GPSIMD -> hH/hV.
  C) Projection streams during the vertical pass: after each vertical
     block, proj for its 16 column-tiles (h^T stripe-seq layout as
     lhsT) + residual add + batched column-strided x loads / out stores.
"""

import math
from contextlib import ExitStack

import numpy as np
import ml_dtypes

import concourse.bass as bass
import concourse.bacc as bacc
import concourse.mybir as mybir
import concourse.tile as tile
from concourse.bass_utils import run_bass_kernel_spmd

F32 = mybir.dt.float32
BF16 = mybir.dt.bfloat16
AF = mybir.ActivationFunctionType
ALU = mybir.AluOpType

_ACT_TABLES_PATCHED = False


def _patch_act_tables():
    """Force every activation onto the natural_log_exp_and_others table set.

    This kernel only uses Exp, Ln and Copy on ScalarE; the default chooser
    maps Exp->exp_and_others and Ln->natural_log, reloading the table RAMs
    (~2.7us) on every alternation. Emptying all other sets (order and thus
    act_func_set_id indices preserved) pins one table load for the whole
    kernel.
    """
    global _ACT_TABLES_PATCHED
    if _ACT_TABLES_PATCHED:
        return
    import concourse.hw_specs as hw_specs
    import concourse.bacc as _bacc_mod
    orig = hw_specs.get_activation_tables

    def patched(module_arch):
        tables = orig(module_arch)
        keep = "natural_log_exp_and_others"
        if keep in tables:
            for name in tables:
                if name != keep:
                    tables[name] = set()
        return tables

    hw_specs.get_activation_tables = patched
    _bacc_mod.get_activation_tables = patched
    _ACT_TABLES_PATCHED = True

B = 8
HH = 128
WW = 128
C = 256
T = HH * WW         # 16384 tokens
NT = T // 128       # 128 token tiles
NS = 64             # stripes per direction
SEQ = 256           # stripe seq len (2 * 128)
NHD = 4             # heads per direction
HD = 32
SCALE = HD ** -0.5
EPS = 1e-5
NB = 8              # stripes per compaction block
GA = 4              # token tiles per x-load group


def build_nc(has_qbias: bool, has_pbias: bool) -> bass.Bass:
    nc = bacc.Bacc("TRN2", target_bir_lowering=False, debug=False)
    x_h = nc.dram_tensor("x", [T, C], F32, kind="ExternalInput")
    wqkv_h = nc.dram_tensor("wqkv", [2, 128, 768], BF16, kind="ExternalInput")
    wproj_h = nc.dram_tensor("wproj", [2, 128, 256], BF16, kind="ExternalInput")
    bqkv_h = nc.dram_tensor("bqkv", [1, 768], BF16, kind="ExternalInput")
    bproj_h = nc.dram_tensor("bproj", [1, 256], BF16, kind="ExternalInput")
    ident_h = nc.dram_tensor("ident", [128, 128], BF16, kind="ExternalInput")
    out_h = nc.dram_tensor("out", [T, C], F32, kind="ExternalOutput")
    # x viewed [i, w, c] for column-strided residual loads (token = i*128+w)
    x_cv = x_h[:, :].rearrange("(i w) c -> i w c", w=WW)
    out_cv = out_h[:, :].rearrange("(i w) c -> i w c", w=WW)

    with tile.TileContext(nc) as tc, tc.tile_pool(name="persist", bufs=1) as pp:
        # ---------------- persistent SBUF ----------------
        ytAB = pp.tile([128, 2, T], BF16, name="ytAB", tag="ytAB")
        hH = pp.tile([128, T], BF16, name="hH", tag="hH")
        hV = pp.tile([128, T], BF16, name="hV", tag="hV")
        wqkv = pp.tile([128, 2 * 768], BF16, name="wqkv", tag="wqkv")
        wproj = pp.tile([128, 2 * 256], BF16, name="wproj", tag="wproj")
        brow = pp.tile([1, 768], BF16, name="brow", tag="brow")
        bprow = pp.tile([1, 256], BF16, name="bprow", tag="bprow")
        ones = pp.tile([1, 256], BF16, name="ones", tag="ones")
        ident = pp.tile([128, 128], BF16, name="ident", tag="ident")
        # manual v ring with persistent ones columns
        v_ring = [pp.tile([128, 2, 4, 64], BF16, name=f"vr{i}", tag=f"vr{i}")
                  for i in range(3)]

        nc.sync.dma_start(out=wqkv[:, 0:768], in_=wqkv_h[0])
        nc.sync.dma_start(out=wqkv[:, 768:1536], in_=wqkv_h[1])
        nc.sync.dma_start(out=wproj[:, 0:256], in_=wproj_h[0])
        nc.sync.dma_start(out=wproj[:, 256:512], in_=wproj_h[1])
        if has_qbias:
            nc.sync.dma_start(out=brow[:], in_=bqkv_h[:])
        if has_pbias:
            nc.sync.dma_start(out=bprow[:], in_=bproj_h[:])
        nc.vector.memset(ones[:], 1.0)
        nc.sync.dma_start(out=ident[:], in_=ident_h[:, :])
        for vr in v_ring:
            nc.vector.memset(vr[:, :, :, 32:64], 1.0)

        ytA = ytAB[:, 0, :]
        ytB = ytAB[:, 1, :]
        # stripe-sliced channel-major views of y^T
        ytAh = ytA.rearrange("p (h w) -> p h w", h=HH)
        ytBh = ytB.rearrange("p (h w) -> p h w", h=HH)
        ytAv = ytA.rearrange("p (h w) -> p w h", h=HH)
        ytBv = ytB.rearrange("p (h w) -> p w h", h=HH)
        # hH viewed for column-tile proj lhsT: col = s*256 + j*128 + w
        hHw = hH[:].rearrange("p (s j w) -> p w s j", j=2, w=WW)

        with ExitStack() as phase_ab:
            # -------- phase A pools --------
            xa_pool = phase_ab.enter_context(tc.tile_pool(name="xa", bufs=2))
            st_pool = phase_ab.enter_context(tc.tile_pool(name="st", bufs=4))
            mv_pool = phase_ab.enter_context(tc.tile_pool(name="mv", bufs=10))
            rb_pool = phase_ab.enter_context(tc.tile_pool(name="rb", bufs=3))
            yt_pool = phase_ab.enter_context(tc.tile_pool(name="yt", bufs=5))
            tp_pool = phase_ab.enter_context(
                tc.tile_pool(name="tp", bufs=1, space="PSUM"))
            # -------- phase B pools --------
            qkv_pool = phase_ab.enter_context(
                tc.tile_pool(name="qkvps", bufs=1, space="PSUM"))
            s_pools = [phase_ab.enter_context(
                tc.tile_pool(name="sps%d" % i, bufs=1, space="PSUM"))
                for i in range(2)]
            o_pool = phase_ab.enter_context(
                tc.tile_pool(name="ops", bufs=1, space="PSUM"))
            qksb_pool = phase_ab.enter_context(tc.tile_pool(name="qksb", bufs=2))
            esb_pools = [phase_ab.enter_context(tc.tile_pool(name="esb%d" % i, bufs=2)) for i in range(2)]
            oall_pool = phase_ab.enter_context(tc.tile_pool(name="oall", bufs=1))
            hst_pool = phase_ab.enter_context(tc.tile_pool(name="hst", bufs=2))
            dst_pool = phase_ab.enter_context(tc.tile_pool(name="dst", bufs=1))
            dr_pool = phase_ab.enter_context(tc.tile_pool(name="dr", bufs=1))

            # ---------------- phase A group ----------------
            def emit_a_group(g):
                # tiles 4g .. 4g+3
                xa = xa_pool.tile([128, GA, 256], F32, tag="xa")
                nc.sync.dma_start(
                    out=xa[:],
                    in_=x_h[:, :].rearrange("(n p) c -> p n c", p=128)
                    [:, 4 * g:4 * g + GA, :])
                varb = rb_pool.tile([128, GA], F32, tag="varb")
                mvs = []
                for k in range(GA):
                    st6 = st_pool.tile([128, 6], F32, tag="st6")
                    nc.vector.bn_stats(st6[:], xa[:, k, :])
                    mv = mv_pool.tile([128, 2], F32, tag="mv")
                    nc.vector.bn_aggr(mv[:], st6[:])
                    mvs.append(mv)
                    nc.vector.tensor_scalar_add(varb[:, k:k + 1], mv[:, 1:2], EPS)
                lnv = rb_pool.tile([128, GA], F32, tag="lnv")
                nc.scalar.activation(lnv[:], varb[:], AF.Ln)
                rstd = rb_pool.tile([128, GA], F32, tag="rstd")
                nc.scalar.activation(rstd[:], lnv[:], AF.Exp, scale=-0.5)
                for pair in range(GA // 2):
                    tp = tp_pool.tile([128, 2, 2, 128], BF16, tag="tp")
                    for k in (2 * pair, 2 * pair + 1):
                        yt_ = yt_pool.tile([128, 256], BF16, tag="yt")
                        nc.vector.tensor_scalar(
                            yt_[:], xa[:, k, :], mvs[k][:, 0:1], rstd[:, k:k + 1],
                            ALU.subtract, ALU.mult)
                        nc.tensor.transpose(
                            tp[:, 0, k - 2 * pair, :], yt_[:, 0:128], ident[:])
                        nc.tensor.transpose(
                            tp[:, 1, k - 2 * pair, :], yt_[:, 128:256], ident[:])
                    i0 = (4 * g + 2 * pair) * 128
                    nc.vector.tensor_copy(
                        ytAB[:, :, i0:i0 + 256].rearrange(
                            "p a (t w) -> p a t w", t=2),
                        tp[:])

            # ---------------- phase B stripe ----------------
            def do_stripe(horiz, g, oall, sidx):
                qoff = 0 if horiz else 128
                yviews = (ytAh, ytBh) if horiz else (ytAv, ytBv)
                rview = [yv[:, 2 * g:2 * g + 2, :] for yv in yviews]
                # ---- qkv: one psum tile [q^T | k^T | v] ----
                qkv_ps = qkv_pool.tile([128, 768], F32, tag="qkvps")
                for kc in range(2):
                    wof = kc * 768
                    last = kc == 1 and not has_qbias
                    nc.tensor.matmul(
                        qkv_ps[:, 0:256], lhsT=wqkv[:, wof + qoff:wof + qoff + 128],
                        rhs=rview[kc], start=kc == 0, stop=last)
                    nc.tensor.matmul(
                        qkv_ps[:, 256:512],
                        lhsT=wqkv[:, wof + 256 + qoff:wof + 384 + qoff],
                        rhs=rview[kc], start=kc == 0, stop=last)
                    for sc in range(2):
                        nc.tensor.matmul(
                            qkv_ps[:, 512 + sc * 128:640 + sc * 128],
                            lhsT=rview[kc][:, sc, :],
                            rhs=wqkv[:, wof + 512 + qoff:wof + 640 + qoff],
                            start=kc == 0, stop=last)
                if has_qbias:
                    nc.tensor.matmul(
                        qkv_ps[:, 0:256], lhsT=brow[:, qoff:qoff + 128],
                        rhs=ones[:, 0:256], start=False, stop=True)
                    nc.tensor.matmul(
                        qkv_ps[:, 256:512], lhsT=brow[:, 256 + qoff:384 + qoff],
                        rhs=ones[:, 0:256], start=False, stop=True)
                    for sc in range(2):
                        nc.tensor.matmul(
                            qkv_ps[:, 512 + sc * 128:640 + sc * 128],
                            lhsT=ones[:, 0:128],
                            rhs=brow[:, 512 + qoff:640 + qoff],
                            start=False, stop=True)
                qk_sb = qksb_pool.tile([128, 512], BF16, tag="qksb")
                nc.vector.tensor_copy(qk_sb[:], qkv_ps[:, 0:512])
                v_sb = v_ring[sidx % 3]
                nc.vector.tensor_copy(
                    v_sb[:, :, :, 0:32],
                    qkv_ps[:, 512:768].rearrange("p (s h d) -> p s h d", s=2, h=4))
                o_ps = o_pool.tile([128, 512], F32, tag="ops")
                for half in range(2):
                    s_ps = s_pools[half].tile([128, 1024], F32, tag="sps%d" % half)
                    for hh in range(2):
                        h = 2 * half + hh
                        for sc in range(2):
                            nc.tensor.matmul(
                                s_ps[:, hh * 512 + sc * 256:hh * 512 + sc * 256 + 256],
                                lhsT=qk_sb[32 * h:32 * h + 32, 256 + sc * 128:384 + sc * 128],
                                rhs=qk_sb[32 * h:32 * h + 32, 0:256],
                                start=True, stop=True,
                                tile_position=(32 * h, 0))
                    e_sb = esb_pools[half].tile([128, 1024], BF16, tag="esb%d" % half)
                    nc.scalar.activation(e_sb[:], s_ps[:], AF.Exp, scale=SCALE)
                    p = half
                    for sc in range(2):
                        h0, h1 = 2 * p, 2 * p + 1
                        nc.tensor.matmul(
                            o_ps[0:64, p * 256:p * 256 + 256],
                            lhsT=v_sb[:, sc, h0, :],
                            rhs=e_sb[:, sc * 256:sc * 256 + 256],
                            start=sc == 0, stop=sc == 1,
                            tile_position=(0, 0))
                        nc.tensor.matmul(
                            o_ps[64:128, p * 256:p * 256 + 256],
                            lhsT=v_sb[:, sc, h1, :],
                            rhs=e_sb[:, 512 + sc * 256:512 + sc * 256 + 256],
                            start=sc == 0, stop=sc == 1,
                            tile_position=(0, 64))
                # ---- evict O (+denoms) into block staging ----
                s_in = g % NB
                nc.scalar.copy(oall[:, s_in, :, :], o_ps[:].rearrange(
                    "p (q n) -> p q n", q=2))

            def finalize_block(hdst, blk, oall):
                # compaction: 8 SWDGE DMAs; band b in {0,1}, col block p in {0,1}
                hstage = hst_pool.tile([128, NB, 256], BF16, tag="hst")
                dstage = dst_pool.tile([128, NB, 256], F32, tag="dst")
                for p in range(2):
                    for b in range(2):
                        q0 = 64 * p + 32 * b
                        nc.gpsimd.dma_start(
                            out=hstage[q0:q0 + 32, :, :],
                            in_=oall[64 * b:64 * b + 32, :, p, :])
                        nc.gpsimd.dma_start(
                            out=dstage[q0:q0 + 32, :, :],
                            in_=oall[64 * b + 32:64 * b + 64, :, p, :])
                drec = dr_pool.tile([128, NB, 256], F32, tag="dr")
                nc.vector.reciprocal_approx_fast(drec[:], dstage[:])
                nc.gpsimd.tensor_tensor(
                    hdst[:, blk * NB * 256:(blk + 1) * NB * 256],
                    hstage[:].rearrange("p a b -> p (a b)"),
                    drec[:].rearrange("p a b -> p (a b)"),
                    ALU.mult)

            # ---------------- A prologue + horizontal ----------------
            emit_a_group(0)
            emit_a_group(1)
            next_g = 2
            oall = None
            for s in range(NS):
                want = min(NT // GA - 1, (2 * s + 1) // GA + 1)
                while next_g <= want:
                    emit_a_group(next_g)
                    next_g += 1
                if s % NB == 0:
                    oall = oall_pool.tile([128, NB, 2, 256], BF16, tag="oall")
                do_stripe(True, s, oall, s)
                if s % NB == NB - 1:
                    finalize_block(hH, s // NB, oall)
            while next_g < NT // GA:
                emit_a_group(next_g)
                next_g += 1

        # ---------------- vertical + streaming proj ----------------
        with ExitStack() as phase_c:
            qkv_pool = phase_c.enter_context(
                tc.tile_pool(name="qkvps2", bufs=1, space="PSUM"))
            s_pools = [phase_c.enter_context(
                tc.tile_pool(name="sps2_%d" % i, bufs=1, space="PSUM"))
                for i in range(2)]
            o_pool = phase_c.enter_context(
                tc.tile_pool(name="ops2", bufs=1, space="PSUM"))
            p_pool = phase_c.enter_context(
                tc.tile_pool(name="pps", bufs=1, space="PSUM"))
            qksb_pool = phase_c.enter_context(tc.tile_pool(name="qksb2", bufs=2))
            esb_pools = [phase_c.enter_context(tc.tile_pool(name="esb2_%d" % i, bufs=2)) for i in range(2)]
            oall_pool = phase_c.enter_context(tc.tile_pool(name="oall2", bufs=1))
            hst_pool = phase_c.enter_context(tc.tile_pool(name="hst2", bufs=2))
            dst_pool = phase_c.enter_context(tc.tile_pool(name="dst2", bufs=1))
            dr_pool = phase_c.enter_context(tc.tile_pool(name="dr2", bufs=1))
            xr_pool = phase_c.enter_context(tc.tile_pool(name="xr", bufs=2))
            po_pool = phase_c.enter_context(tc.tile_pool(name="po", bufs=2))

            def do_stripe2(horiz, g, oall, sidx,
                           qkv_pool=qkv_pool, s_pools=s_pools, o_pool=o_pool,
                           qksb_pool=qksb_pool, esb_pools=esb_pools):
                qoff = 128
                yviews = (ytAv, ytBv)
                rview = [yv[:, 2 * g:2 * g + 2, :] for yv in yviews]
                qkv_ps = qkv_pool.tile([128, 768], F32, tag="qkvps2")
                for kc in range(2):
                    wof = kc * 768
                    last = kc == 1 and not has_qbias
                    nc.tensor.matmul(
                        qkv_ps[:, 0:256], lhsT=wqkv[:, wof + qoff:wof + qoff + 128],
                        rhs=rview[kc], start=kc == 0, stop=last)
                    nc.tensor.matmul(
                        qkv_ps[:, 256:512],
                        lhsT=wqkv[:, wof + 256 + qoff:wof + 384 + qoff],
                        rhs=rview[kc], start=kc == 0, stop=last)
                    for sc in range(2):
                        nc.tensor.matmul(
                            qkv_ps[:, 512 + sc * 128:640 + sc * 128],
                            lhsT=rview[kc][:, sc, :],
                            rhs=wqkv[:, wof + 512 + qoff:wof + 640 + qoff],
                            start=kc == 0, stop=last)
                if has_qbias:
                    nc.tensor.matmul(
                        qkv_ps[:, 0:256], lhsT=brow[:, qoff:qoff + 128],
                        rhs=ones[:, 0:256], start=False, stop=True)
                    nc.tensor.matmul(
                        qkv_ps[:, 256:512], lhsT=brow[:, 256 + qoff:384 + qoff],
                        rhs=ones[:, 0:256], start=False, stop=True)
                    for sc in range(2):
                        nc.tensor.matmul(
                            qkv_ps[:, 512 + sc * 128:640 + sc * 128],
                            lhsT=ones[:, 0:128],
                            rhs=brow[:, 512 + qoff:640 + qoff],
                            start=False, stop=True)
                qk_sb = qksb_pool.tile([128, 512], BF16, tag="qksb2")
                nc.vector.tensor_copy(qk_sb[:], qkv_ps[:, 0:512])
                v_sb = v_ring[sidx % 3]
                nc.vector.tensor_copy(
                    v_sb[:, :, :, 0:32],
                    qkv_ps[:, 512:768].rearrange("p (s h d) -> p s h d", s=2, h=4))
                o_ps = o_pool.tile([128, 512], F32, tag="ops2")
                for half in range(2):
                    s_ps = s_pools[half].tile([128, 1024], F32, tag="sps2_%d" % half)
                    for hh in range(2):
                        h = 2 * half + hh
                        for sc in range(2):
                            nc.tensor.matmul(
                                s_ps[:, hh * 512 + sc * 256:hh * 512 + sc * 256 + 256],
                                lhsT=qk_sb[32 * h:32 * h + 32, 256 + sc * 128:384 + sc * 128],
                                rhs=qk_sb[32 * h:32 * h + 32, 0:256],
                                start=True, stop=True,
                                tile_position=(32 * h, 0))
                    e_sb = esb_pools[half].tile([128, 1024], BF16, tag="esb2_%d" % half)
                    nc.scalar.activation(e_sb[:], s_ps[:], AF.Exp, scale=SCALE)
                    p = half
                    for sc in range(2):
                        h0, h1 = 2 * p, 2 * p + 1
                        nc.tensor.matmul(
                            o_ps[0:64, p * 256:p * 256 + 256],
                            lhsT=v_sb[:, sc, h0, :],
                            rhs=e_sb[:, sc * 256:sc * 256 + 256],
                            start=sc == 0, stop=sc == 1,
                            tile_position=(0, 0))
                        nc.tensor.matmul(
                            o_ps[64:128, p * 256:p * 256 + 256],
                            lhsT=v_sb[:, sc, h1, :],
                            rhs=e_sb[:, 512 + sc * 256:512 + sc * 256 + 256],
                            start=sc == 0, stop=sc == 1,
                            tile_position=(0, 64))
                s_in = g % NB
                nc.scalar.copy(oall[:, s_in, :, :], o_ps[:].rearrange(
                    "p (q n) -> p q n", q=2))

            def finalize_block2(hdst, blk, oall,
                                hst_pool=hst_pool, dst_pool=dst_pool,
                                dr_pool=dr_pool):
                hstage = hst_pool.tile([128, NB, 256], BF16, tag="hst2")
                dstage = dst_pool.tile([128, NB, 256], F32, tag="dst2")
                for p in range(2):
                    for b in range(2):
                        q0 = 64 * p + 32 * b
                        nc.gpsimd.dma_start(
                            out=hstage[q0:q0 + 32, :, :],
                            in_=oall[64 * b:64 * b + 32, :, p, :])
                        nc.gpsimd.dma_start(
                            out=dstage[q0:q0 + 32, :, :],
                            in_=oall[64 * b + 32:64 * b + 64, :, p, :])
                drec = dr_pool.tile([128, NB, 256], F32, tag="dr2")
                nc.vector.reciprocal_approx_fast(drec[:], dstage[:])
                nc.gpsimd.tensor_tensor(
                    hdst[:, blk * NB * 256:(blk + 1) * NB * 256],
                    hstage[:].rearrange("p a b -> p (a b)"),
                    drec[:].rearrange("p a b -> p (a b)"),
                    ALU.mult)

            GP = 2  # column tiles per proj group

            def proj_group(w0):
                xr = xr_pool.tile([128, GP, 256], F32, tag="xr")
                nc.sync.dma_start(out=xr[:], in_=x_cv[:, w0:w0 + GP, :])
                po = po_pool.tile([128, GP, 256], F32, tag="po")
                for k in range(GP):
                    w = w0 + k
                    p_ps = p_pool.tile([128, 256], F32, tag="pps")
                    nc.tensor.matmul(
                        p_ps[:], lhsT=hHw[:, w, :, :],
                        rhs=wproj[:, 0:256], start=True, stop=False)
                    nc.tensor.matmul(
                        p_ps[:], lhsT=hV[:, (w // 2) * 256 + (w % 2) * 128:
                                        (w // 2) * 256 + (w % 2) * 128 + 128],
                        rhs=wproj[:, 256:512], start=False, stop=not has_pbias)
                    if has_pbias:
                        nc.tensor.matmul(
                            p_ps[:], lhsT=ones[:, 0:128], rhs=bprow[:],
                            start=False, stop=True)
                    nc.vector.tensor_add(po[:, k, :], p_ps[:], xr[:, k, :])
                nc.sync.dma_start(out=out_cv[:, w0:w0 + GP, :], in_=po[:])

            # proj for block b-1 is spread across the stripes of block b to
            # keep PE fed while ACT runs exp
            oall = None
            for s in range(NS):
                if s % NB == 0:
                    oall = oall_pool.tile([128, NB, 2, 256], BF16, tag="oall2")
                do_stripe2(False, s, oall, s)
                blk, j = s // NB, s % NB
                if j == NB - 1:
                    finalize_block2(hV, blk, oall)
                if blk > 0:
                    proj_group((blk - 1) * 2 * NB + j * GP)
            for grp in range(2 * NB // GP):
                proj_group((NS // NB - 1) * 2 * NB + grp * GP)

    return nc


_NC_CACHE = {}


def _get_nc(has_qbias, has_pbias):
    key = (has_qbias, has_pbias)
    if key not in _NC_CACHE:
        _patch_act_tables()
        nc = build_nc(has_qbias, has_pbias)
        nc.finalize()
        _NC_CACHE[key] = nc
    return _NC_CACHE[key]


def kernel(x, Wqkv, bqkv, Wproj, bproj, gamma, beta, _trace=False):
    x = np.asarray(x, np.float32)
    Wqkv = np.asarray(Wqkv, np.float32)
    bqkv = np.asarray(bqkv, np.float32)
    Wproj = np.asarray(Wproj, np.float32)
    bproj = np.asarray(bproj, np.float32)
    gamma = np.asarray(gamma, np.float32)
    beta = np.asarray(beta, np.float32)

    Wg = gamma[:, None] * Wqkv                      # fold LN affine scale
    bq = beta @ Wqkv + bqkv                         # fold LN affine shift
    has_qbias = bool(np.any(bq != 0.0))
    has_pbias = bool(np.any(bproj != 0.0))

    bf = ml_dtypes.bfloat16
    wqkv_np = np.ascontiguousarray(Wg.reshape(2, 128, 768)).astype(bf)
    wproj_np = np.ascontiguousarray(Wproj.reshape(2, 128, 256)).astype(bf)
    bq_np = bq.reshape(1, 768).astype(bf)
    bp_np = bproj.reshape(1, 256).astype(bf)

    nc = _get_nc(has_qbias, has_pbias)
    in_maps = []
    for b in range(B):
        in_maps.append({
            "x": np.ascontiguousarray(x[b].reshape(T, C)),
            "wqkv": wqkv_np, "wproj": wproj_np,
            "bqkv": bq_np, "bproj": bp_np,
            "ident": np.eye(128, dtype=np.float32).astype(bf),
        })
    res = run_bass_kernel_spmd(nc, in_maps, list(range(B)), trace=_trace)
    out = np.stack([np.asarray(res.results[b]["out"]).reshape(HH, WW, C)
                    for b in range(B)])
    if _trace:
        return out.astype(np.float32), res
    return out.astype(np.float32)


# revision 16
# speedup vs baseline: 1.2116x; 1.2116x over previous
"""CSWin self-attention Trainium2 kernel (optimized).

Sharding: data-parallel over batch B=8 across 8 cores (1 image per core).

Per-core pipeline (image = 128x128 spatial, C=256):
  A) LayerNorm (gamma/beta folded into Wqkv on host): batched x loads
     (4 token-tiles per DMA), bn_stats/aggr per tile, rstd via ACT
     ln/exp (same table set as attention's exp -> no table reloads),
     apply on DVE, PE-transpose to channel-major y^T, paired evac.
  B) Horizontal stripes overlap phase A (stripe s needs row tiles
     2s,2s+1 only); vertical stripes after.  The stripe loop is
     software-pipelined for the FIFO PE queue: emit S(s), exp(s),
     qkv(s+1), AV(s) so qkv of the next stripe executes during exp
     and the PE never idles long enough to re-throttle (HAM).
     Per stripe (seq 256, 4 heads x head_dim 32):
       qkv matmuls into one PSUM tile (q^T,k^T ch-major; v token-major),
       S^T row-tiled 4 heads (K=32), exp on ACT (scale folded),
       attn@V col-tiled head pairs (M=64 with ones cols -> denominators
       in rows 32/96), o evac to a per-8-stripe staging block.
     Per 8-stripe block: 8 SWDGE (gpsimd) compaction DMAs with 3D APs
     (4 for O^T bands, 4 for denominator bands), reciprocal on DVE,
     normalize multiply on GPSIMD -> hH/hV.
  C) Projection streams during the vertical pass (2 column-tiles per
     stripe, one block behind) + residual add + batched column-strided
     x loads / out stores.
"""

from contextlib import ExitStack

import numpy as np
import ml_dtypes

import concourse.bass as bass
import concourse.bacc as bacc
import concourse.mybir as mybir
import concourse.tile as tile
from concourse.bass_utils import run_bass_kernel_spmd

F32 = mybir.dt.float32
BF16 = mybir.dt.bfloat16
AF = mybir.ActivationFunctionType
ALU = mybir.AluOpType

B = 8
HH = 128
WW = 128
C = 256
T = HH * WW         # 16384 tokens
NT = T // 128       # 128 token tiles
NS = 64             # stripes per direction
NHD = 4             # heads per direction
HD = 32
SCALE = HD ** -0.5
EPS = 1e-5
NB = 8              # stripes per compaction block
GA = 4              # token tiles per x-load group
GP = 2              # column tiles per proj group

_ACT_TABLES_PATCHED = False


def _patch_act_tables():
    """Force every activation onto the natural_log_exp_and_others set.

    This kernel only uses Exp, Ln and Copy on ScalarE; the default chooser
    maps Exp->exp_and_others and Ln->natural_log, reloading the ACT table
    RAMs (~2.7us) on every alternation. Emptying all other sets (order and
    thus act_func_set_id indices preserved) pins one table load total.
    """
    global _ACT_TABLES_PATCHED
    if _ACT_TABLES_PATCHED:
        return
    import concourse.hw_specs as hw_specs
    import concourse.bacc as _bacc_mod
    orig = hw_specs.get_activation_tables

    def patched(module_arch):
        tables = orig(module_arch)
        keep = "natural_log_exp_and_others"
        if keep in tables:
            for name in tables:
                if name != keep:
                    tables[name] = set()
        return tables

    hw_specs.get_activation_tables = patched
    _bacc_mod.get_activation_tables = patched
    _ACT_TABLES_PATCHED = True


def build_nc(has_qbias: bool, has_pbias: bool) -> bass.Bass:
    nc = bacc.Bacc("TRN2", target_bir_lowering=False, debug=False)
    x_h = nc.dram_tensor("x", [T, C], F32, kind="ExternalInput")
    wqkv_h = nc.dram_tensor("wqkv", [2, 128, 768], BF16, kind="ExternalInput")
    wproj_h = nc.dram_tensor("wproj", [2, 128, 256], BF16, kind="ExternalInput")
    bqkv_h = nc.dram_tensor("bqkv", [1, 768], BF16, kind="ExternalInput")
    bproj_h = nc.dram_tensor("bproj", [1, 256], BF16, kind="ExternalInput")
    ident_h = nc.dram_tensor("ident", [128, 128], BF16, kind="ExternalInput")
    out_h = nc.dram_tensor("out", [T, C], F32, kind="ExternalOutput")
    x_tv = x_h[:, :].rearrange("(n p) c -> p n c", p=128)
    x_cv = x_h[:, :].rearrange("(i w) c -> i w c", w=WW)
    out_cv = out_h[:, :].rearrange("(i w) c -> i w c", w=WW)

    with tile.TileContext(nc) as tc, tc.tile_pool(name="persist", bufs=1) as pp:
        # ---------------- persistent SBUF ----------------
        ytAB = pp.tile([128, 2, T], BF16, name="ytAB", tag="ytAB")
        hH = pp.tile([128, T], BF16, name="hH", tag="hH")
        hV = pp.tile([128, T], BF16, name="hV", tag="hV")
        wqkv = pp.tile([128, 2 * 768], BF16, name="wqkv", tag="wqkv")
        wproj = pp.tile([128, 2 * 256], BF16, name="wproj", tag="wproj")
        brow = pp.tile([1, 768], BF16, name="brow", tag="brow")
        bprow = pp.tile([1, 256], BF16, name="bprow", tag="bprow")
        ones = pp.tile([1, 256], BF16, name="ones", tag="ones")
        ident = pp.tile([128, 128], BF16, name="ident", tag="ident")
        v_ring = [pp.tile([128, 2, 4, 64], BF16, name=f"vr{i}", tag=f"vr{i}")
                  for i in range(3)]

        nc.sync.dma_start(out=wqkv[:, 0:768], in_=wqkv_h[0])
        nc.sync.dma_start(out=wqkv[:, 768:1536], in_=wqkv_h[1])
        nc.sync.dma_start(out=wproj[:, 0:256], in_=wproj_h[0])
        nc.sync.dma_start(out=wproj[:, 256:512], in_=wproj_h[1])
        if has_qbias:
            nc.sync.dma_start(out=brow[:], in_=bqkv_h[:])
        if has_pbias:
            nc.sync.dma_start(out=bprow[:], in_=bproj_h[:])
        nc.vector.memset(ones[:], 1.0)
        nc.sync.dma_start(out=ident[:], in_=ident_h[:, :])
        for vr in v_ring:
            nc.vector.memset(vr[:, :, :, 32:64], 1.0)

        ytA = ytAB[:, 0, :]
        ytB = ytAB[:, 1, :]
        ytAh = ytA.rearrange("p (h w) -> p h w", h=HH)
        ytBh = ytB.rearrange("p (h w) -> p h w", h=HH)
        ytAv = ytA.rearrange("p (h w) -> p w h", h=HH)
        ytBv = ytB.rearrange("p (h w) -> p w h", h=HH)
        hHw = hH[:].rearrange("p (s j w) -> p w s j", j=2, w=WW)

        def make_pools(stk, tag, with_tp, with_proj):
            p = {}
            p['qkv'] = stk.enter_context(
                tc.tile_pool(name=f"qkv{tag}", bufs=1, space="PSUM"))
            p['s'] = stk.enter_context(
                tc.tile_pool(name=f"s{tag}", bufs=1, space="PSUM"))
            p['o'] = stk.enter_context(
                tc.tile_pool(name=f"o{tag}", bufs=1, space="PSUM"))
            if with_tp:
                p['tp'] = stk.enter_context(
                    tc.tile_pool(name=f"tp{tag}", bufs=1, space="PSUM"))
            if with_proj:
                p['pp'] = stk.enter_context(
                    tc.tile_pool(name=f"pp{tag}", bufs=1, space="PSUM"))
            p['qksb'] = stk.enter_context(tc.tile_pool(name=f"qksb{tag}", bufs=3))
            p['esb'] = stk.enter_context(tc.tile_pool(name=f"esb{tag}", bufs=2))
            p['oall'] = stk.enter_context(tc.tile_pool(name=f"oall{tag}", bufs=1))
            p['hst'] = stk.enter_context(tc.tile_pool(name=f"hst{tag}", bufs=2))
            p['dst'] = stk.enter_context(tc.tile_pool(name=f"dst{tag}", bufs=1))
            p['dr'] = stk.enter_context(tc.tile_pool(name=f"dr{tag}", bufs=1))
            return p

        def qkv_part(P, tag, qoff, yviews, g, sidx):
            """qkv matmuls + evacuation; returns (qk_sb, v_sb)."""
            rview = [yv[:, 2 * g:2 * g + 2, :] for yv in yviews]
            qkv_ps = P['qkv'].tile([128, 768], F32, tag=f"qkv{tag}")
            for kc in range(2):
                wof = kc * 768
                last = kc == 1 and not has_qbias
                nc.tensor.matmul(
                    qkv_ps[:, 0:256], lhsT=wqkv[:, wof + qoff:wof + qoff + 128],
                    rhs=rview[kc], start=kc == 0, stop=last)
                nc.tensor.matmul(
                    qkv_ps[:, 256:512],
                    lhsT=wqkv[:, wof + 256 + qoff:wof + 384 + qoff],
                    rhs=rview[kc], start=kc == 0, stop=last)
                for sc in range(2):
                    nc.tensor.matmul(
                        qkv_ps[:, 512 + sc * 128:640 + sc * 128],
                        lhsT=rview[kc][:, sc, :],
                        rhs=wqkv[:, wof + 512 + qoff:wof + 640 + qoff],
                        start=kc == 0, stop=last)
            if has_qbias:
                nc.tensor.matmul(
                    qkv_ps[:, 0:256], lhsT=brow[:, qoff:qoff + 128],
                    rhs=ones[:, 0:256], start=False, stop=True)
                nc.tensor.matmul(
                    qkv_ps[:, 256:512], lhsT=brow[:, 256 + qoff:384 + qoff],
                    rhs=ones[:, 0:256], start=False, stop=True)
                for sc in range(2):
                    nc.tensor.matmul(
                        qkv_ps[:, 512 + sc * 128:640 + sc * 128],
                        lhsT=ones[:, 0:128],
                        rhs=brow[:, 512 + qoff:640 + qoff],
                        start=False, stop=True)
            qk_sb = P['qksb'].tile([128, 512], BF16, tag=f"qksb{tag}")
            nc.vector.tensor_copy(qk_sb[:], qkv_ps[:, 0:512])
            v_sb = v_ring[sidx % 3]
            nc.vector.tensor_copy(
                v_sb[:, :, :, 0:32],
                qkv_ps[:, 512:768].rearrange("p (s h d) -> p s h d", s=2, h=4))
            return qk_sb, v_sb

        def s_part(P, tag, qk_sb):
            """S^T row-tiled matmuls + exp; returns e_sb."""
            s_ps = P['s'].tile([128, 2048], F32, tag=f"s{tag}")
            for h in range(NHD):
                for sc in range(2):
                    nc.tensor.matmul(
                        s_ps[:, h * 512 + sc * 256:h * 512 + sc * 256 + 256],
                        lhsT=qk_sb[32 * h:32 * h + 32,
                                   256 + sc * 128:384 + sc * 128],
                        rhs=qk_sb[32 * h:32 * h + 32, 0:256],
                        start=True, stop=True,
                        tile_position=(32 * h, 0))
            e_sb = P['esb'].tile([128, 2048], BF16, tag=f"esb{tag}")
            nc.scalar.activation(e_sb[:], s_ps[:], AF.Exp, scale=SCALE)
            return e_sb

        def av_part(P, tag, v_sb, e_sb, oall, s_in):
            """attn@V col-tiled + O/denominator eviction into block staging."""
            o_ps = P['o'].tile([128, 512], F32, tag=f"o{tag}")
            for p in range(2):
                for sc in range(2):
                    h0, h1 = 2 * p, 2 * p + 1
                    nc.tensor.matmul(
                        o_ps[0:64, p * 256:p * 256 + 256],
                        lhsT=v_sb[:, sc, h0, :],
                        rhs=e_sb[:, h0 * 512 + sc * 256:h0 * 512 + sc * 256 + 256],
                        start=sc == 0, stop=sc == 1,
                        tile_position=(0, 0))
                    nc.tensor.matmul(
                        o_ps[64:128, p * 256:p * 256 + 256],
                        lhsT=v_sb[:, sc, h1, :],
                        rhs=e_sb[:, h1 * 512 + sc * 256:h1 * 512 + sc * 256 + 256],
                        start=sc == 0, stop=sc == 1,
                        tile_position=(0, 64))
            nc.scalar.copy(oall[:, s_in, :, :], o_ps[:].rearrange(
                "p (q n) -> p q n", q=2))

        def finalize_block(P, tag, hdst, blk, oall):
            hstage = P['hst'].tile([128, NB, 256], BF16, tag=f"hst{tag}")
            dstage = P['dst'].tile([128, NB, 256], F32, tag=f"dst{tag}")
            for p in range(2):
                for b in range(2):
                    q0 = 64 * p + 32 * b
                    nc.gpsimd.dma_start(
                        out=hstage[q0:q0 + 32, :, :],
                        in_=oall[64 * b:64 * b + 32, :, p, :])
                    nc.gpsimd.dma_start(
                        out=dstage[q0:q0 + 32, :, :],
                        in_=oall[64 * b + 32:64 * b + 64, :, p, :])
            drec = P['dr'].tile([128, NB, 256], F32, tag=f"dr{tag}")
            nc.vector.reciprocal_approx_fast(drec[:], dstage[:])
            nc.gpsimd.tensor_tensor(
                hdst[:, blk * NB * 256:(blk + 1) * NB * 256],
                hstage[:].rearrange("p a b -> p (a b)"),
                drec[:].rearrange("p a b -> p (a b)"),
                ALU.mult)

        # ================ phase A + horizontal ================
        with ExitStack() as stk:
            xa_pool = stk.enter_context(tc.tile_pool(name="xa", bufs=2))
            st_pool = stk.enter_context(tc.tile_pool(name="st", bufs=4))
            mv_pool = stk.enter_context(tc.tile_pool(name="mv", bufs=10))
            rb_pool = stk.enter_context(tc.tile_pool(name="rb", bufs=3))
            yt_pool = stk.enter_context(tc.tile_pool(name="yt", bufs=5))
            P = make_pools(stk, "h", with_tp=True, with_proj=False)

            def emit_a_group(g):
                xa = xa_pool.tile([128, GA, 256], F32, tag="xa")
                nc.sync.dma_start(out=xa[:], in_=x_tv[:, 4 * g:4 * g + GA, :])
                varb = rb_pool.tile([128, GA], F32, tag="varb")
                mvs = []
                for k in range(GA):
                    st6 = st_pool.tile([128, 6], F32, tag="st6")
                    nc.vector.bn_stats(st6[:], xa[:, k, :])
                    mv = mv_pool.tile([128, 2], F32, tag="mv")
                    nc.vector.bn_aggr(mv[:], st6[:])
                    mvs.append(mv)
                    nc.vector.tensor_scalar_add(varb[:, k:k + 1], mv[:, 1:2], EPS)
                lnv = rb_pool.tile([128, GA], F32, tag="lnv")
                nc.scalar.activation(lnv[:], varb[:], AF.Ln)
                rstd = rb_pool.tile([128, GA], F32, tag="rstd")
                nc.scalar.activation(rstd[:], lnv[:], AF.Exp, scale=-0.5)
                for pair in range(GA // 2):
                    tp = P['tp'].tile([128, 2, 2, 128], BF16, tag="tp")
                    for k in (2 * pair, 2 * pair + 1):
                        yt_ = yt_pool.tile([128, 256], BF16, tag="yt")
                        nc.vector.tensor_scalar(
                            yt_[:], xa[:, k, :], mvs[k][:, 0:1],
                            rstd[:, k:k + 1], ALU.subtract, ALU.mult)
                        nc.tensor.transpose(
                            tp[:, 0, k - 2 * pair, :], yt_[:, 0:128], ident[:])
                        nc.tensor.transpose(
                            tp[:, 1, k - 2 * pair, :], yt_[:, 128:256], ident[:])
                    i0 = (4 * g + 2 * pair) * 128
                    nc.vector.tensor_copy(
                        ytAB[:, :, i0:i0 + 256].rearrange(
                            "p a (t w) -> p a t w", t=2),
                        tp[:])

            yv_h = (ytAh, ytBh)
            emit_a_group(0)
            emit_a_group(1)
            next_g = 2
            oall = P['oall'].tile([128, NB, 2, 256], BF16, tag="oallh")
            qkv_next = qkv_part(P, "h", 0, yv_h, 0, 0)
            for s in range(NS):
                qk_sb, v_sb = qkv_next
                e_sb = s_part(P, "h", qk_sb)
                # next stripe's qkv (and phase A work) fills the exp window
                want = min(NT // GA - 1, (2 * s + 1) // GA + 1)
                while next_g <= want:
                    emit_a_group(next_g)
                    next_g += 1
                if s + 1 < NS:
                    qkv_next = qkv_part(P, "h", 0, yv_h, s + 1, s + 1)
                av_part(P, "h", v_sb, e_sb, oall, s % NB)
                if s % NB == NB - 1:
                    finalize_block(P, "h", hH, s // NB, oall)
                    if s + 1 < NS:
                        oall = P['oall'].tile(
                            [128, NB, 2, 256], BF16, tag="oallh")
            while next_g < NT // GA:
                emit_a_group(next_g)
                next_g += 1

        # ================ vertical + streaming proj ================
        with ExitStack() as stk:
            P = make_pools(stk, "v", with_tp=False, with_proj=True)
            xr_pool = stk.enter_context(tc.tile_pool(name="xr", bufs=3))
            po_pool = stk.enter_context(tc.tile_pool(name="po", bufs=3))

            def proj_group(w0):
                xr = xr_pool.tile([128, GP, 256], F32, tag="xr")
                nc.sync.dma_start(out=xr[:], in_=x_cv[:, w0:w0 + GP, :])
                po = po_pool.tile([128, GP, 256], F32, tag="po")
                for k in range(GP):
                    w = w0 + k
                    p_ps = P['pp'].tile([128, 256], F32, tag="ppv")
                    nc.tensor.matmul(
                        p_ps[:], lhsT=hHw[:, w, :, :],
                        rhs=wproj[:, 0:256], start=True, stop=False)
                    nc.tensor.matmul(
                        p_ps[:], lhsT=hV[:, (w // 2) * 256 + (w % 2) * 128:
                                        (w // 2) * 256 + (w % 2) * 128 + 128],
                        rhs=wproj[:, 256:512], start=False, stop=not has_pbias)
                    if has_pbias:
                        nc.tensor.matmul(
                            p_ps[:], lhsT=ones[:, 0:128], rhs=bprow[:],
                            start=False, stop=True)
                    nc.vector.tensor_add(po[:, k, :], p_ps[:], xr[:, k, :])
                nc.sync.dma_start(out=out_cv[:, w0:w0 + GP, :], in_=po[:])

            yv_v = (ytAv, ytBv)
            oall = P['oall'].tile([128, NB, 2, 256], BF16, tag="oallv")
            qkv_next = qkv_part(P, "v", 128, yv_v, 0, 0)
            for s in range(NS):
                qk_sb, v_sb = qkv_next
                e_sb = s_part(P, "v", qk_sb)
                if s + 1 < NS:
                    qkv_next = qkv_part(P, "v", 128, yv_v, s + 1, s + 1)
                blk, j = s // NB, s % NB
                # proj for block blk-1 spread across this block's stripes
                if blk > 0:
                    proj_group((blk - 1) * 2 * NB + j * GP)
                av_part(P, "v", v_sb, e_sb, oall, j)
                if j == NB - 1:
                    finalize_block(P, "v", hV, blk, oall)
                    if s + 1 < NS:
                        oall = P['oall'].tile(
                            [128, NB, 2, 256], BF16, tag="oallv")
            for grp in range(2 * NB // GP):
                proj_group((NS // NB - 1) * 2 * NB + grp * GP)

    return nc


_NC_CACHE = {}


def _get_nc(has_qbias, has_pbias):
    key = (has_qbias, has_pbias)
    if key not in _NC_CACHE:
        _patch_act_tables()
        nc = build_nc(has_qbias, has_pbias)
        nc.finalize()
        _NC_CACHE[key] = nc
    return _NC_CACHE[key]


def kernel(x, Wqkv, bqkv, Wproj, bproj, gamma, beta, _trace=False):
    x = np.asarray(x, np.float32)
    Wqkv = np.asarray(Wqkv, np.float32)
    bqkv = np.asarray(bqkv, np.float32)
    Wproj = np.asarray(Wproj, np.float32)
    bproj = np.asarray(bproj, np.float32)
    gamma = np.asarray(gamma, np.float32)
    beta = np.asarray(beta, np.float32)

    Wg = gamma[:, None] * Wqkv                      # fold LN affine scale
    bq = beta @ Wqkv + bqkv                         # fold LN affine shift
    has_qbias = bool(np.any(bq != 0.0))
    has_pbias = bool(np.any(bproj != 0.0))

    bf = ml_dtypes.bfloat16
    wqkv_np = np.ascontiguousarray(Wg.reshape(2, 128, 768)).astype(bf)
    wproj_np = np.ascontiguousarray(Wproj.reshape(2, 128, 256)).astype(bf)
    bq_np = bq.reshape(1, 768).astype(bf)
    bp_np = bproj.reshape(1, 256).astype(bf)

    nc = _get_nc(has_qbias, has_pbias)
    in_maps = []
    for b in range(B):
        in_maps.append({
            "x": np.ascontiguousarray(x[b].reshape(T, C)),
            "wqkv": wqkv_np, "wproj": wproj_np,
            "bqkv": bq_np, "bproj": bp_np,
            "ident": np.eye(128, dtype=np.float32).astype(bf),
        })
    res = run_bass_kernel_spmd(nc, in_maps, list(range(B)), trace=_trace)
    out = np.stack([np.asarray(res.results[b]["out"]).reshape(HH, WW, C)
                    for b in range(B)])
    if _trace:
        return out.astype(np.float32), res
    return out.astype(np.float32)
